# revision 21
# baseline (speedup 1.0000x reference)
"""Dense causal MHA (B=2, S=2048, H=16, D=128, hidden=2048) on 8 Trainium2 cores.

Sharding: data-parallel over batch (2) x tensor-parallel over head groups
(4 heads/core).  Core c handles batch c//4, heads 4*(c%4) .. 4*(c%4)+3.
Each core computes a partial output (its heads' contribution to the out
projection, with bo folded in on host); the host sums the 4 partials per batch.

v2 design notes (vs the f32r baseline):
  - x^T, all weight matrices, V and the exp'd attention weights are bf16
    (halves DMA + SBUF; matmuls stay 1 cycle/row); scores path (K, q)
    stays f32r for accuracy.
  - x^T is DMA'd once into SBUF and stays resident for both the K/V
    projections and the Q projection (no phase-2 reload).
  - softmax denominators are partition-reductions on the otherwise-idle
    GpSimd/Pool engine (tensor_reduce axis=C), not PE ones-matmuls.
  - the attention inner loop is software-pipelined (scores run LAG tiles
    ahead of the context matmuls) and out-projection (chunk j-1) +
    Q-projection (chunk j+1) matmuls are interleaved into the attention
    stream as PE filler, so the in-order PE never waits on the Act
    engine's exp and the p-state stays at full clock.
"""

import sys

sys.path.insert(0, "/opt/trn_rl_repo")

from contextlib import ExitStack

import numpy as np
import ml_dtypes

import concourse.tile as tile
from concourse import bacc, mybir
from concourse.bass_utils import run_bass_kernel_spmd

S = 2048
HID = 2048
D = 128
LH = 4            # heads per core
DL = LH * D       # 512 local inner dims
SC = 512          # q/s chunk
NSC = S // SC     # 4
HCH = HID // 128  # 16 contraction chunks
N_CORES = 8
LAG = 2           # context matmul lag behind scores in the attn pipeline

f32 = mybir.dt.float32
f32r = mybir.dt.float32r
bf16 = mybir.dt.bfloat16
Exp = mybir.ActivationFunctionType.Exp
Ident = mybir.ActivationFunctionType.Identity
AxC = mybir.AxisListType.C
Add = mybir.AluOpType.add

_CACHE = {}


def _build_nc():
    nc = bacc.Bacc("TRN2", target_bir_lowering=False, debug=False,
                   num_devices=N_CORES)

    def din(name, shape, dt):
        return nc.dram_tensor(name, shape, dt, kind="ExternalInput").ap()

    xT = din("xT", [HID, S], bf16)
    wqT = din("wqT", [HID, DL], bf16)
    wkT = din("wkT", [HID, DL], bf16)
    wvT = din("wvT", [HID, DL], bf16)
    woT = din("woT", [DL, HID], bf16)
    bq2 = din("bq2", [128, LH], f32)
    bk2 = din("bk2", [128, LH], f32)
    cosT = din("cosT", [128, S], f32)
    sinT = din("sinT", [128, S], f32)
    trib = din("trib", [128, 128], bf16)
    onec = din("onec", [128, 1], bf16)
    out = nc.dram_tensor("out", [S, HID], bf16, kind="ExternalOutput").ap()

    with tile.TileContext(nc) as tc, ExitStack() as ctx:
        P = ctx.enter_context(tc.tile_pool(name="persist", bufs=1))
        tri_sb = P.tile([128, 128], bf16, tag="tri")
        bq_sb = P.tile([128, LH], f32, tag="bq")
        bk_sb = P.tile([128, LH], f32, tag="bk")
        onec_sb = P.tile([128, 1], bf16, tag="onec")

        x_sb = [P.tile([128, S], bf16, tag=f"x{h}", name=f"xsb{h}")
                for h in range(HCH)]
        K_sb = [P.tile([128, S], f32r, tag=f"K{d}", name=f"Ksb{d}")
                for d in range(LH)]
        V_sb = [P.tile([128, DL], bf16, tag=f"V{t}", name=f"Vsb{t}")
                for t in range(S // 128)]
        wq_sb = [P.tile([128, DL], bf16, tag=f"wq{h}", name=f"wqsb{h}")
                 for h in range(HCH)]
        cos_sb = P.tile([128, S], f32, tag="cos")
        sin_sb = P.tile([128, S], f32, tag="sin")

        def rope(pool, raw, dst, sl):
            """dst = raw*cos + rotate_half(raw)*sin; the rotate is folded
            into partition-offset reads against a half-sign-flipped sin
            table (sin_sb rows >=64 carry the minus sign)."""
            m1 = pool.tile([128, SC], f32, tag="rm1", bufs=2)
            nc.vector.tensor_mul(m1[:], raw[:], cos_sb[:, sl])
            m2 = pool.tile([128, SC], f32, tag="rm2", bufs=2)
            nc.vector.tensor_mul(m2[0:64, :], raw[64:128, :],
                                 sin_sb[64:128, sl])
            nc.vector.tensor_mul(m2[64:128, :], raw[0:64, :],
                                 sin_sb[0:64, sl])
            nc.vector.tensor_add(dst, m1[:], m2[:])

        # ---- phase 1: K^T (roped, f32r) and V (bf16) for the whole seq ----
        with tc.tile_pool(name="p1w", bufs=1) as WP, \
             tc.tile_pool(name="p1t", bufs=1) as TP, \
             tc.tile_pool(name="p1ps", bufs=2, space="PSUM") as PK, \
             tc.tile_pool(name="p1pv", bufs=2, space="PSUM") as PV:
            wk_sb = [WP.tile([128, DL], bf16, tag=f"wk{h}", name=f"wksb{h}")
                     for h in range(HCH)]
            wv_sb = [WP.tile([128, DL], bf16, tag=f"wv{h}", name=f"wvsb{h}")
                     for h in range(HCH)]

            # DMA order = critical path order: K-path weights + first x
            # chunk, then wv (V blocks run after K blocks in j0), rope
            # tables, rest of x, then wq (phase 2).
            for h in range(HCH):
                nc.sync.dma_start(wk_sb[h][:], wkT[128 * h:128 * (h + 1), :])
                nc.sync.dma_start(x_sb[h][:, 0:SC],
                                  xT[128 * h:128 * (h + 1), 0:SC])
            for h in range(HCH):
                nc.sync.dma_start(wv_sb[h][:], wvT[128 * h:128 * (h + 1), :])
            nc.sync.dma_start(bk_sb[:], bk2[:])
            nc.sync.dma_start(cos_sb[:], cosT[:])
            nc.sync.dma_start(sin_sb[:], sinT[:])
            nc.sync.dma_start(tri_sb[:], trib[:])
            nc.sync.dma_start(bq_sb[:], bq2[:])
            nc.sync.dma_start(onec_sb[:], onec[:])
            for h in range(HCH):
                nc.sync.dma_start(x_sb[h][:, SC:S],
                                  xT[128 * h:128 * (h + 1), SC:S])
            for h in range(HCH):
                nc.sync.dma_start(wq_sb[h][:], wqT[128 * h:128 * (h + 1), :])
            for j in range(NSC):
                sl = slice(SC * j, SC * (j + 1))
                for d in range(LH):
                    psk = PK.tile([128, SC], f32, tag="pk",
                                  name=f"psk{j}_{d}")
                    for h in range(HCH):
                        nc.tensor.matmul(psk[:],
                                         wk_sb[h][:, 128 * d:128 * (d + 1)],
                                         x_sb[h][:, sl], start=(h == 0),
                                         stop=(h == HCH - 1))
                    kraw = TP.tile([128, SC], f32, tag="kraw", bufs=2)
                    nc.scalar.activation(kraw[:], psk[:], Ident,
                                         bias=bk_sb[:, d:d + 1], scale=1.0)
                    rope(TP, kraw, K_sb[d][:, sl], sl)
                for st in range(4):
                    psv = PV.tile([128, DL], f32, tag="pv",
                                  name=f"psv{j}_{st}")
                    for h in range(HCH):
                        nc.tensor.matmul(
                            psv[:],
                            x_sb[h][:, SC * j + 128 * st:SC * j + 128 * (st + 1)],
                            wv_sb[h][:], start=(h == 0), stop=(h == HCH - 1))
                    nc.scalar.copy(V_sb[4 * j + st][:], psv[:])

        # ---- phase 2: per q-chunk: Q proj + rope, attention, out proj ----
        with tc.tile_pool(name="p2t", bufs=1) as T2, \
             tc.tile_pool(name="p2q", bufs=1) as QP, \
             tc.tile_pool(name="p2ex", bufs=1) as EX, \
             tc.tile_pool(name="p2dn", bufs=1) as DN, \
             tc.tile_pool(name="p2rc", bufs=1) as RC, \
             tc.tile_pool(name="p2rb", bufs=1) as RB, \
             tc.tile_pool(name="p2cu", bufs=1) as CU, \
             tc.tile_pool(name="p2ct", bufs=1) as CT, \
             tc.tile_pool(name="p2wo", bufs=1) as WO, \
             tc.tile_pool(name="p2ot", bufs=1) as OT, \
             tc.tile_pool(name="p2pa", bufs=1, space="PSUM") as PA, \
             tc.tile_pool(name="p2ps", bufs=2, space="PSUM") as PS, \
             tc.tile_pool(name="p2pc", bufs=2, space="PSUM") as PC, \
             tc.tile_pool(name="p2pd", bufs=1, space="PSUM") as PD, \
             tc.tile_pool(name="p2po", bufs=2, space="PSUM") as PO:

            q_tiles = {}   # j -> [q_sb tile per d]

            def gen_qproj(jq):
                """Yield one closure per PE matmul for the Q projection of
                chunk jq; Act bias + DVE rope ride on the last unit of
                each d block."""
                sl = slice(SC * jq, SC * (jq + 1))
                q_tiles[jq] = [QP.tile([128, SC], f32r, tag=f"q{d}", bufs=2,
                                       name=f"qro{jq}_{d}")
                               for d in range(LH)]
                for d in range(LH):
                    psq = PA.tile([128, SC], f32, tag="pA",
                                  name=f"psq{jq}_{d}")

                    def unit(h, d=d, psq=psq):
                        nc.tensor.matmul(psq[:],
                                         wq_sb[h][:, 128 * d:128 * (d + 1)],
                                         x_sb[h][:, sl], start=(h == 0),
                                         stop=(h == HCH - 1))
                        if h == HCH - 1:
                            qraw = T2.tile([128, SC], f32, tag="qraw", bufs=1)
                            nc.scalar.activation(qraw[:], psq[:], Ident,
                                                 bias=bq_sb[:, d:d + 1],
                                                 scale=1.0)
                            rope(T2, qraw, q_tiles[jq][d][:], sl)
                    for h in range(HCH):
                        yield lambda h=h: unit(h)

            def gen_outproj(jo, cts):
                """Yield one closure per PE matmul for the out projection of
                chunk jo (uses normalized ct tiles cts); wo DMA is issued a
                group ahead; psum drain alternates DVE/Pool."""
                wo_t = {}

                def issue_dma(oc):
                    osl = slice(SC * oc, SC * (oc + 1))
                    wo_t[oc] = [WO.tile([128, SC], bf16, tag="wo", bufs=8,
                                        name=f"wot{jo}_{oc}_{it}")
                                for it in range(LH)]
                    for it in range(LH):
                        nc.sync.dma_start(wo_t[oc][it][:],
                                          woT[128 * it:128 * (it + 1), osl])

                issue_dma(0)
                yield lambda: None  # prefetch-only unit: wo DMA in flight
                for oc in range(4):
                    osl = slice(SC * oc, SC * (oc + 1))
                    if oc + 1 < 4:
                        issue_dma(oc + 1)
                    for qt in range(4):
                        pso = PO.tile([128, SC], f32, tag="po",
                                      name=f"pso{jo}_{oc}_{qt}")

                        def unit(it, oc=oc, qt=qt, pso=pso, osl=osl):
                            nc.tensor.matmul(
                                pso[:],
                                cts[it][:, 128 * qt:128 * (qt + 1)],
                                wo_t[oc][it][:], start=(it == 0),
                                stop=(it == LH - 1))
                            if it == LH - 1:
                                ot = OT.tile([128, SC], bf16, tag="ot",
                                             bufs=2, name=f"ot{jo}_{oc}_{qt}")
                                if qt % 2 == 0:
                                    nc.vector.tensor_copy(ot[:], pso[:])
                                else:
                                    nc.scalar.copy(ot[:], pso[:])
                                nc.sync.dma_start(
                                    out[SC * jo + 128 * qt:
                                        SC * jo + 128 * (qt + 1), osl], ot[:])
                        for it in range(LH):
                            yield lambda it=it: unit(it)

            def pull(filler, n=1):
                for _ in range(n):
                    u = next(filler, None)
                    if u is None:
                        return False
                    u()
                return True

            # Q projection for chunk 0 runs as a plain block.
            for u in gen_qproj(0):
                u()

            def chain(*gens):
                for g in gens:
                    if g is not None:
                        yield from g

            pending_ct = None  # ct tiles of chunk j-1 (consumed by filler)
            for j in range(NSC):
                T = 4 * j + 4
                filler = chain(
                    gen_outproj(j - 1, pending_ct) if j > 0 else None,
                    gen_qproj(j + 1) if j + 1 < NSC else None)
                # uniform filler pacing: spread the supply over the 4*T
                # attention steps so the stream neither dumps at the chunk
                # boundary nor starves the chunk tail
                supply = (65 if j > 0 else 0) + (64 if j + 1 < NSC else 0)
                rate = supply / (4 * T)
                acc = 0.0
                ct = [None] * LH

                for h in range(LH):
                    psc = PC.tile([128, SC], f32, tag="pc",
                                  name=f"psc{j}_{h}")
                    pd = PD.tile([1, SC], f32, tag="pd", name=f"pd{j}_{h}")
                    exs = [None] * T
                    css = [None] * T

                    def ctx_mm(t, psc=psc, h=h, T=T):
                        nc.tensor.matmul(psc[:, css[t]],
                                         V_sb[t][:, 128 * h:128 * (h + 1)],
                                         exs[t][:, css[t]], start=(t == 0),
                                         stop=(t == T - 1))

                    for t in range(T):
                        p = t - 4 * j  # >=0 for diagonal tiles
                        c0 = 128 * p if p > 0 else 0
                        cs = slice(c0, SC)
                        css[t] = cs
                        pss = PS.tile([128, SC], f32, tag="ps")
                        nc.tensor.matmul(pss[:, cs],
                                         K_sb[h][:, 128 * t:128 * (t + 1)],
                                         q_tiles[j][h][:, cs],
                                         start=True, stop=True)
                        ex = EX.tile([128, SC], bf16, tag="ex", bufs=3)
                        exs[t] = ex
                        nc.scalar.activation(ex[:, cs], pss[:, cs], Exp)
                        if p >= 0:
                            dsl = slice(128 * p, 128 * (p + 1))
                            nc.vector.tensor_mul(ex[:, dsl], ex[:, dsl],
                                                 tri_sb[:])
                        nc.tensor.matmul(pd[:, cs], onec_sb[:], ex[:, cs],
                                         start=(t == 0), stop=(t == T - 1))
                        if t >= LAG:
                            ctx_mm(t - LAG)
                        acc += rate
                        if acc >= 1.0:
                            n = int(acc)
                            pull(filler, n)
                            acc -= n
                    for t in range(max(T - LAG, 0), T):
                        ctx_mm(t)
                    # normalization (PE-free): drain den + context, then
                    # reciprocal -> partition-broadcast -> scale
                    ctu = CU.tile([128, SC], bf16, tag=f"cu{h}", bufs=1,
                                  name=f"ctu{j}_{h}")
                    nc.vector.tensor_copy(ctu[:], psc[:])
                    den = DN.tile([1, SC], f32, tag="dh", bufs=2,
                                  name=f"den{j}_{h}")
                    nc.vector.tensor_copy(den[:], pd[:])
                    rec = RC.tile([1, SC], f32, tag="rec", bufs=2,
                                  name=f"rec{j}_{h}")
                    nc.vector.reciprocal_approx_fast(out=rec[:], in_=den[:])
                    rb = RB.tile([128, SC], f32, tag="rb", bufs=1,
                                 name=f"rb{j}_{h}")
                    nc.gpsimd.partition_broadcast(rb[:], rec[:])
                    cth = CT.tile([128, SC], bf16, tag=f"ct{h}", bufs=2,
                                  name=f"ct{j}_{h}")
                    nc.vector.tensor_mul(cth[:], ctu[:], rb[:])
                    ct[h] = cth
                while pull(filler):
                    pass
                pending_ct = list(ct)
            for u in gen_outproj(NSC - 1, pending_ct):
                u()
    nc.compile()
    return nc


def _get_nc():
    if "nc" not in _CACHE:
        _CACHE["nc"] = _build_nc()
    return _CACHE["nc"]


def _consts():
    if "consts" not in _CACHE:
        inv = (10000.0 ** (-np.arange(0, D, 2, dtype=np.float64) / D))
        t = np.arange(S, dtype=np.float64)
        fr = np.outer(t, inv)                      # [S, 64]
        cos = np.concatenate([np.cos(fr)] * 2, 1).T.astype(np.float32)
        sin = np.concatenate([np.sin(fr)] * 2, 1).T.astype(np.float32)
        sin[64:] *= -1.0
        tri = (np.arange(128)[:, None] <= np.arange(128)[None, :])
        _CACHE["consts"] = {
            "cosT": np.ascontiguousarray(cos),
            "sinT": np.ascontiguousarray(sin),
            "trib": np.ascontiguousarray(
                tri.astype(ml_dtypes.bfloat16)),
            "onec": np.ones((128, 1), ml_dtypes.bfloat16),
        }
    return _CACHE["consts"]


def _marshal(hidden_states, Wq, bq, Wk, bk, Wv, bv, Wo, bo):
    consts = _consts()
    scale = 1.0 / np.sqrt(D)
    bf = ml_dtypes.bfloat16
    xTs = [np.ascontiguousarray(hidden_states[b].T.astype(bf))
           for b in range(2)]
    in_maps = []
    for c in range(N_CORES):
        b, hg = c // 4, c % 4
        rows = slice(DL * hg, DL * (hg + 1))
        m = dict(consts)
        m["xT"] = xTs[b]
        m["wqT"] = np.ascontiguousarray((Wq[rows] * scale).T.astype(bf))
        m["wkT"] = np.ascontiguousarray(Wk[rows].T.astype(bf))
        m["wvT"] = np.ascontiguousarray(Wv[rows].T.astype(bf))
        m["woT"] = np.ascontiguousarray(Wo[:, rows].T.astype(bf))
        m["bq2"] = np.ascontiguousarray(
            (bq[rows] * scale).reshape(LH, 128).T.astype(np.float32))
        m["bk2"] = np.ascontiguousarray(
            bk[rows].reshape(LH, 128).T.astype(np.float32))
        in_maps.append(m)
    return in_maps


def _gather(results, bias):
    out = np.empty((2, S, HID), np.float32)
    for b in range(2):
        acc = results[4 * b]["out"].astype(np.float32)
        for g in range(1, 4):
            acc = acc + results[4 * b + g]["out"].astype(np.float32)
        out[b] = acc + bias
    return out


def _run(inputs, **kw):
    nc = _get_nc()
    in_maps = _marshal(**{k: np.asarray(v) for k, v in inputs.items()})
    return run_bass_kernel_spmd(nc, in_maps, core_ids=list(range(N_CORES)),
                                **kw)


def _host_bias(inputs):
    Wo = np.asarray(inputs["Wo"], np.float64)
    bv = np.asarray(inputs["bv"], np.float64)
    bo = np.asarray(inputs["bo"], np.float64)
    return (bo + Wo @ bv).astype(np.float32)


def kernel(**inputs):
    res = _run(inputs)
    return _gather(res.results, _host_bias(inputs))


def kernel_traced(**inputs):
    """Like kernel() but with NTFF profiling; returns (output, results)."""
    import types

    try:
        import antenv.axon_hooks  # noqa: F401
    except ImportError:
        from trn_agent_boot.trn_boot import _ntff_profile_via_ctypes
        hook = _ntff_profile_via_ctypes("/opt/axon/libaxon_pjrt.so")
        mod = types.ModuleType("antenv.axon_hooks")
        mod.get_axon_ntff_profile_hook = lambda: hook
        mod.set_axon_ntff_profile_hook = lambda h: None
        sys.modules["antenv.axon_hooks"] = mod
    res = _run(inputs, trace=True)
    return _gather(res.results, _host_bias(inputs)), res


# revision 22
# speedup vs baseline: 1.1723x; 1.1723x over previous
"""Dense causal MHA (B=2, S=2048, H=16, D=128, hidden=2048) on 8 Trainium2 cores.

Sharding: data-parallel over batch (2) x tensor-parallel over head groups
(4 heads/core).  Core c handles batch c//4, heads 4*(c%4) .. 4*(c%4)+3.
Each core computes a partial output (its heads' contribution to the out
projection, with bo folded in on host); the host sums the 4 partials per batch.

v2 design notes (vs the f32r baseline):
  - x^T, all weight matrices, V and the exp'd attention weights are bf16
    (halves DMA + SBUF; matmuls stay 1 cycle/row); scores path (K, q)
    stays f32r for accuracy.
  - x^T is DMA'd once into SBUF and stays resident for both the K/V
    projections and the Q projection (no phase-2 reload).
  - softmax denominators are partition-reductions on the otherwise-idle
    GpSimd/Pool engine (tensor_reduce axis=C), not PE ones-matmuls.
  - the attention inner loop is software-pipelined (scores run LAG tiles
    ahead of the context matmuls) and out-projection (chunk j-1) +
    Q-projection (chunk j+1) matmuls are interleaved into the attention
    stream as PE filler, so the in-order PE never waits on the Act
    engine's exp and the p-state stays at full clock.
"""

import sys

sys.path.insert(0, "/opt/trn_rl_repo")

from contextlib import ExitStack

import numpy as np
import ml_dtypes

import concourse.tile as tile
from concourse import bacc, mybir
from concourse.bass_utils import run_bass_kernel_spmd

S = 2048
HID = 2048
D = 128
LH = 4            # heads per core
DL = LH * D       # 512 local inner dims
SC = 512          # q/s chunk
NSC = S // SC     # 4
HCH = HID // 128  # 16 contraction chunks
N_CORES = 8
LAG = 2           # context matmul lag behind scores in the attn pipeline

f32 = mybir.dt.float32
f32r = mybir.dt.float32r
bf16 = mybir.dt.bfloat16
Exp = mybir.ActivationFunctionType.Exp
Ident = mybir.ActivationFunctionType.Identity
AxC = mybir.AxisListType.C
Add = mybir.AluOpType.add

_CACHE = {}


def _build_nc():
    nc = bacc.Bacc("TRN2", target_bir_lowering=False, debug=False,
                   num_devices=N_CORES)

    def din(name, shape, dt):
        return nc.dram_tensor(name, shape, dt, kind="ExternalInput").ap()

    xT = din("xT", [HID, S], bf16)
    wqT = din("wqT", [HID, DL], bf16)
    wkT = din("wkT", [HID, DL], bf16)
    wvT = din("wvT", [HID, DL], bf16)
    woT = din("woT", [DL, HID], bf16)
    bq2 = din("bq2", [128, LH], f32)
    bk2 = din("bk2", [128, LH], f32)
    cosT = din("cosT", [128, S], f32)
    sinT = din("sinT", [128, S], f32)
    trib = din("trib", [128, 128], bf16)
    onec = din("onec", [128, 1], bf16)
    out = nc.dram_tensor("out", [S, HID], bf16, kind="ExternalOutput").ap()

    with tile.TileContext(nc) as tc, ExitStack() as ctx:
        P = ctx.enter_context(tc.tile_pool(name="persist", bufs=1))
        tri_sb = P.tile([128, 128], bf16, tag="tri")
        bq_sb = P.tile([128, LH], f32, tag="bq")
        bk_sb = P.tile([128, LH], f32, tag="bk")
        onec_sb = P.tile([128, 1], bf16, tag="onec")

        x_sb = [P.tile([128, S], bf16, tag=f"x{h}", name=f"xsb{h}")
                for h in range(HCH)]
        K_sb = [P.tile([128, S], f32r, tag=f"K{d}", name=f"Ksb{d}")
                for d in range(LH)]
        V_sb = [P.tile([128, DL], bf16, tag=f"V{t}", name=f"Vsb{t}")
                for t in range(S // 128)]
        wq_sb = [P.tile([128, DL], bf16, tag=f"wq{h}", name=f"wqsb{h}")
                 for h in range(HCH)]
        cos_sb = P.tile([128, S], f32, tag="cos")
        sin_sb = P.tile([128, S], f32, tag="sin")

        def rope(pool, raw, dst, sl):
            """dst = raw*cos + rotate_half(raw)*sin; the rotate is folded
            into partition-offset reads against a half-sign-flipped sin
            table (sin_sb rows >=64 carry the minus sign)."""
            m1 = pool.tile([128, SC], f32, tag="rm1", bufs=2)
            nc.vector.tensor_mul(m1[:], raw[:], cos_sb[:, sl])
            m2 = pool.tile([128, SC], f32, tag="rm2", bufs=2)
            nc.vector.tensor_mul(m2[0:64, :], raw[64:128, :],
                                 sin_sb[64:128, sl])
            nc.vector.tensor_mul(m2[64:128, :], raw[0:64, :],
                                 sin_sb[0:64, sl])
            nc.vector.tensor_add(dst, m1[:], m2[:])

        # ---- phase 1: K^T (roped, f32r) and V (bf16) for the whole seq ----
        with tc.tile_pool(name="p1w", bufs=1) as WP, \
             tc.tile_pool(name="p1t", bufs=1) as TP, \
             tc.tile_pool(name="p1ps", bufs=2, space="PSUM") as PK, \
             tc.tile_pool(name="p1pv", bufs=2, space="PSUM") as PV:
            wk_sb = [WP.tile([128, DL], bf16, tag=f"wk{h}", name=f"wksb{h}")
                     for h in range(HCH)]
            wv_sb = [WP.tile([128, DL], bf16, tag=f"wv{h}", name=f"wvsb{h}")
                     for h in range(HCH)]

            # DMA order = critical path order: K-path weights + first x
            # chunk, then wv (V blocks run after K blocks in j0), rope
            # tables, rest of x, then wq (phase 2).
            for h in range(HCH):
                nc.sync.dma_start(wk_sb[h][:], wkT[128 * h:128 * (h + 1), :])
                nc.sync.dma_start(x_sb[h][:, 0:SC],
                                  xT[128 * h:128 * (h + 1), 0:SC])
            for h in range(HCH):
                nc.sync.dma_start(wv_sb[h][:], wvT[128 * h:128 * (h + 1), :])
            nc.sync.dma_start(bk_sb[:], bk2[:])
            nc.sync.dma_start(cos_sb[:], cosT[:])
            nc.sync.dma_start(sin_sb[:], sinT[:])
            nc.sync.dma_start(tri_sb[:], trib[:])
            nc.sync.dma_start(bq_sb[:], bq2[:])
            nc.sync.dma_start(onec_sb[:], onec[:])
            for h in range(HCH):
                nc.sync.dma_start(x_sb[h][:, SC:S],
                                  xT[128 * h:128 * (h + 1), SC:S])
            for h in range(HCH):
                nc.sync.dma_start(wq_sb[h][:], wqT[128 * h:128 * (h + 1), :])
            for j in range(NSC):
                sl = slice(SC * j, SC * (j + 1))
                for d in range(LH):
                    psk = PK.tile([128, SC], f32, tag="pk",
                                  name=f"psk{j}_{d}")
                    for h in range(HCH):
                        nc.tensor.matmul(psk[:],
                                         wk_sb[h][:, 128 * d:128 * (d + 1)],
                                         x_sb[h][:, sl], start=(h == 0),
                                         stop=(h == HCH - 1))
                    kraw = TP.tile([128, SC], f32, tag="kraw", bufs=2)
                    nc.scalar.activation(kraw[:], psk[:], Ident,
                                         bias=bk_sb[:, d:d + 1], scale=1.0)
                    rope(TP, kraw, K_sb[d][:, sl], sl)
                for st in range(4):
                    psv = PV.tile([128, DL], f32, tag="pv",
                                  name=f"psv{j}_{st}")
                    for h in range(HCH):
                        nc.tensor.matmul(
                            psv[:],
                            x_sb[h][:, SC * j + 128 * st:SC * j + 128 * (st + 1)],
                            wv_sb[h][:], start=(h == 0), stop=(h == HCH - 1))
                    nc.scalar.copy(V_sb[4 * j + st][:], psv[:])

        # ---- phase 2: per q-chunk: Q proj + rope, attention, out proj ----
        with tc.tile_pool(name="p2t", bufs=1) as T2, \
             tc.tile_pool(name="p2q", bufs=1) as QP, \
             tc.tile_pool(name="p2ex", bufs=1) as EX, \
             tc.tile_pool(name="p2dn", bufs=1) as DN, \
             tc.tile_pool(name="p2rc", bufs=1) as RC, \
             tc.tile_pool(name="p2rb", bufs=1) as RB, \
             tc.tile_pool(name="p2cu", bufs=1) as CU, \
             tc.tile_pool(name="p2ct", bufs=1) as CT, \
             tc.tile_pool(name="p2wo", bufs=1) as WO, \
             tc.tile_pool(name="p2ot", bufs=1) as OT, \
             tc.tile_pool(name="p2pa", bufs=1, space="PSUM") as PA, \
             tc.tile_pool(name="p2ps", bufs=2, space="PSUM") as PS, \
             tc.tile_pool(name="p2pc", bufs=2, space="PSUM") as PC, \
             tc.tile_pool(name="p2pd", bufs=1, space="PSUM") as PD, \
             tc.tile_pool(name="p2po", bufs=2, space="PSUM") as PO:

            q_tiles = {}   # j -> [q_sb tile per d]

            def gen_qproj(jq):
                """Yield one closure per PE matmul for the Q projection of
                chunk jq; Act bias + DVE rope ride on the last unit of
                each d block."""
                sl = slice(SC * jq, SC * (jq + 1))
                q_tiles[jq] = [QP.tile([128, SC], f32r, tag=f"q{d}", bufs=2,
                                       name=f"qro{jq}_{d}")
                               for d in range(LH)]
                for d in range(LH):
                    psq = PA.tile([128, SC], f32, tag="pA",
                                  name=f"psq{jq}_{d}")

                    def unit(h, d=d, psq=psq):
                        nc.tensor.matmul(psq[:],
                                         wq_sb[h][:, 128 * d:128 * (d + 1)],
                                         x_sb[h][:, sl], start=(h == 0),
                                         stop=(h == HCH - 1))
                        if h == HCH - 1:
                            qraw = T2.tile([128, SC], f32, tag="qraw", bufs=1)
                            nc.scalar.activation(qraw[:], psq[:], Ident,
                                                 bias=bq_sb[:, d:d + 1],
                                                 scale=1.0)
                            rope(T2, qraw, q_tiles[jq][d][:], sl)
                    for h in range(HCH):
                        yield lambda h=h: unit(h)

            def gen_outproj(jo, cts):
                """Yield one closure per PE matmul for the out projection of
                chunk jo (uses normalized ct tiles cts); wo DMA is issued a
                group ahead; psum drain alternates DVE/Pool."""
                wo_t = {}

                def issue_dma(oc):
                    osl = slice(SC * oc, SC * (oc + 1))
                    wo_t[oc] = [WO.tile([128, SC], bf16, tag="wo", bufs=8,
                                        name=f"wot{jo}_{oc}_{it}")
                                for it in range(LH)]
                    for it in range(LH):
                        nc.sync.dma_start(wo_t[oc][it][:],
                                          woT[128 * it:128 * (it + 1), osl])

                issue_dma(0)
                yield lambda: None  # prefetch-only unit: wo DMA in flight
                for oc in range(4):
                    osl = slice(SC * oc, SC * (oc + 1))
                    if oc + 1 < 4:
                        issue_dma(oc + 1)
                    for qt in range(4):
                        pso = PO.tile([128, SC], f32, tag="po",
                                      name=f"pso{jo}_{oc}_{qt}")

                        def unit(it, oc=oc, qt=qt, pso=pso, osl=osl):
                            nc.tensor.matmul(
                                pso[:],
                                cts[it][:, 128 * qt:128 * (qt + 1)],
                                wo_t[oc][it][:], start=(it == 0),
                                stop=(it == LH - 1))
                            if it == LH - 1:
                                ot = OT.tile([128, SC], bf16, tag="ot",
                                             bufs=2, name=f"ot{jo}_{oc}_{qt}")
                                if qt % 2 == 0:
                                    nc.vector.tensor_copy(ot[:], pso[:])
                                else:
                                    nc.scalar.copy(ot[:], pso[:])
                                nc.sync.dma_start(
                                    out[SC * jo + 128 * qt:
                                        SC * jo + 128 * (qt + 1), osl], ot[:])
                        for it in range(LH):
                            yield lambda it=it: unit(it)

            def pull(filler, n=1):
                for _ in range(n):
                    u = next(filler, None)
                    if u is None:
                        return False
                    u()
                return True

            # Q projection for chunk 0 runs as a plain block.
            for u in gen_qproj(0):
                u()

            def chain(*gens):
                for g in gens:
                    if g is not None:
                        yield from g

            pending_ct = None  # ct tiles of chunk j-1 (consumed by filler)
            for j in range(NSC):
                T = 4 * j + 4
                # Q projection first: its Act/DVE tail (bias + rope) must
                # land mid-chunk, not after the j-end filler dump; the out
                # projection has no downstream latency chain so it can trail
                filler = chain(
                    gen_qproj(j + 1) if j + 1 < NSC else None,
                    gen_outproj(j - 1, pending_ct) if j > 0 else None)
                last = j == NSC - 1
                ct = [None] * LH

                for h in range(LH):
                    psc = PC.tile([128, SC], f32, tag="pc",
                                  name=f"psc{j}_{h}")
                    pd = PD.tile([1, SC], f32, tag="pd", name=f"pd{j}_{h}")
                    exs = [None] * T
                    css = [None] * T

                    def ctx_mm(t, psc=psc, h=h, T=T):
                        nc.tensor.matmul(psc[:, css[t]],
                                         V_sb[t][:, 128 * h:128 * (h + 1)],
                                         exs[t][:, css[t]], start=(t == 0),
                                         stop=(t == T - 1))

                    for t in range(T):
                        p = t - 4 * j  # >=0 for diagonal tiles
                        c0 = 128 * p if p > 0 else 0
                        cs = slice(c0, SC)
                        css[t] = cs
                        pss = PS.tile([128, SC], f32, tag="ps")
                        nc.tensor.matmul(pss[:, cs],
                                         K_sb[h][:, 128 * t:128 * (t + 1)],
                                         q_tiles[j][h][:, cs],
                                         start=True, stop=True)
                        ex = EX.tile([128, SC], bf16, tag="ex", bufs=3)
                        exs[t] = ex
                        nc.scalar.activation(ex[:, cs], pss[:, cs], Exp)
                        if p >= 0:
                            dsl = slice(128 * p, 128 * (p + 1))
                            nc.vector.tensor_mul(ex[:, dsl], ex[:, dsl],
                                                 tri_sb[:])
                        nc.tensor.matmul(pd[:, cs], onec_sb[:], ex[:, cs],
                                         start=(t == 0), stop=(t == T - 1))
                        if t >= LAG:
                            ctx_mm(t - LAG)
                        pull(filler)
                    for t in range(max(T - LAG, 0), T):
                        ctx_mm(t)
                    # normalization (PE-free): drain den + context, then
                    # reciprocal -> partition-broadcast -> scale
                    ctu = CU.tile([128, SC], bf16, tag=f"cu{h}", bufs=1,
                                  name=f"ctu{j}_{h}")
                    nc.vector.tensor_copy(ctu[:], psc[:])
                    den = DN.tile([1, SC], f32, tag="dh", bufs=2,
                                  name=f"den{j}_{h}")
                    nc.vector.tensor_copy(den[:], pd[:])
                    rec = RC.tile([1, SC], f32, tag="rec", bufs=2,
                                  name=f"rec{j}_{h}")
                    nc.vector.reciprocal_approx_fast(out=rec[:], in_=den[:])
                    rb = RB.tile([128, SC], f32, tag="rb", bufs=1,
                                 name=f"rb{j}_{h}")
                    nc.gpsimd.partition_broadcast(rb[:], rec[:])
                    cth = CT.tile([128, SC], bf16, tag=f"ct{h}", bufs=2,
                                  name=f"ct{j}_{h}")
                    nc.vector.tensor_mul(cth[:], ctu[:], rb[:])
                    ct[h] = cth
                    if not last:
                        pull(filler, 2)
                while pull(filler):
                    pass
                pending_ct = list(ct)
            for u in gen_outproj(NSC - 1, pending_ct):
                u()
    nc.compile()
    return nc


def _get_nc():
    if "nc" not in _CACHE:
        _CACHE["nc"] = _build_nc()
    return _CACHE["nc"]


def _consts():
    if "consts" not in _CACHE:
        inv = (10000.0 ** (-np.arange(0, D, 2, dtype=np.float64) / D))
        t = np.arange(S, dtype=np.float64)
        fr = np.outer(t, inv)                      # [S, 64]
        cos = np.concatenate([np.cos(fr)] * 2, 1).T.astype(np.float32)
        sin = np.concatenate([np.sin(fr)] * 2, 1).T.astype(np.float32)
        sin[64:] *= -1.0
        tri = (np.arange(128)[:, None] <= np.arange(128)[None, :])
        _CACHE["consts"] = {
            "cosT": np.ascontiguousarray(cos),
            "sinT": np.ascontiguousarray(sin),
            "trib": np.ascontiguousarray(
                tri.astype(ml_dtypes.bfloat16)),
            "onec": np.ones((128, 1), ml_dtypes.bfloat16),
        }
    return _CACHE["consts"]


def _marshal(hidden_states, Wq, bq, Wk, bk, Wv, bv, Wo, bo):
    consts = _consts()
    scale = 1.0 / np.sqrt(D)
    bf = ml_dtypes.bfloat16
    xTs = [np.ascontiguousarray(hidden_states[b].T.astype(bf))
           for b in range(2)]
    in_maps = []
    for c in range(N_CORES):
        b, hg = c // 4, c % 4
        rows = slice(DL * hg, DL * (hg + 1))
        m = dict(consts)
        m["xT"] = xTs[b]
        m["wqT"] = np.ascontiguousarray((Wq[rows] * scale).T.astype(bf))
        m["wkT"] = np.ascontiguousarray(Wk[rows].T.astype(bf))
        m["wvT"] = np.ascontiguousarray(Wv[rows].T.astype(bf))
        m["woT"] = np.ascontiguousarray(Wo[:, rows].T.astype(bf))
        m["bq2"] = np.ascontiguousarray(
            (bq[rows] * scale).reshape(LH, 128).T.astype(np.float32))
        m["bk2"] = np.ascontiguousarray(
            bk[rows].reshape(LH, 128).T.astype(np.float32))
        in_maps.append(m)
    return in_maps


def _gather(results, bias):
    out = np.empty((2, S, HID), np.float32)
    for b in range(2):
        acc = results[4 * b]["out"].astype(np.float32)
        for g in range(1, 4):
            acc = acc + results[4 * b + g]["out"].astype(np.float32)
        out[b] = acc + bias
    return out


def _run(inputs, **kw):
    nc = _get_nc()
    in_maps = _marshal(**{k: np.asarray(v) for k, v in inputs.items()})
    return run_bass_kernel_spmd(nc, in_maps, core_ids=list(range(N_CORES)),
                                **kw)


def _host_bias(inputs):
    Wo = np.asarray(inputs["Wo"], np.float64)
    bv = np.asarray(inputs["bv"], np.float64)
    bo = np.asarray(inputs["bo"], np.float64)
    return (bo + Wo @ bv).astype(np.float32)


def kernel(**inputs):
    res = _run(inputs)
    return _gather(res.results, _host_bias(inputs))


def kernel_traced(**inputs):
    """Like kernel() but with NTFF profiling; returns (output, results)."""
    import types

    try:
        import antenv.axon_hooks  # noqa: F401
    except ImportError:
        from trn_agent_boot.trn_boot import _ntff_profile_via_ctypes
        hook = _ntff_profile_via_ctypes("/opt/axon/libaxon_pjrt.so")
        mod = types.ModuleType("antenv.axon_hooks")
        mod.get_axon_ntff_profile_hook = lambda: hook
        mod.set_axon_ntff_profile_hook = lambda h: None
        sys.modules["antenv.axon_hooks"] = mod
    res = _run(inputs, trace=True)
    return _gather(res.results, _host_bias(inputs)), res
